# revision 22
# baseline (speedup 1.0000x reference)
"""Trainium2 Bass kernel for a dense transformer decoder layer.

Reference shapes: x [2, 2048, 1024], H=16 heads, d_head 64, d_ffn 4096.

Sharding: 8 cores = 2 batches x 4 causal query-chunks of 512 tokens.
Each core redundantly computes K/V for a 2048-slot key buffer holding its
(zero-padded) causal prefix plus its own chunk, so no collectives are
needed. Invalid key slots self-mask: their x columns are zero, so V rows
and the softmax-denominator "ones" column are zero and they contribute
nothing to attention. The own-chunk keys always sit in the last 4
key-tiles, so one shared relative causal mask handles the diagonal and
the program is identical (SPMD) on all cores.

Main matmuls run in bf16 (fp32 accumulate); the softmax-denominator
broadcast runs in fp32r.
"""
import numpy as np
import ml_dtypes

import concourse.bacc as bacc
import concourse.tile as tile
from concourse import mybir
from concourse.bass_utils import run_bass_kernel_spmd

F32 = mybir.dt.float32
F32R = mybir.dt.float32r
BF16 = mybir.dt.bfloat16
AF = mybir.ActivationFunctionType
ALU = mybir.AluOpType

B, L, D = 2, 2048, 1024
H, DH = 16, 64
DFF = 4096
Q = 512          # query tokens per core
P = 2048         # padded key slots per core
NCORES = 8
EPS = 1e-6
NEG = -1.0e5     # additive pre-scale mask; exp(0.125 * NEG) == 0.0


def _build_nc():
    nc = bacc.Bacc()

    xt = nc.dram_tensor("xt", [D, P], BF16, kind="ExternalInput")
    xc = nc.dram_tensor("xc", [Q, D], F32, kind="ExternalInput")
    onesg = nc.dram_tensor("onesg", [P, H], BF16, kind="ExternalInput")
    wq_re = nc.dram_tensor("wq_re", [128, 8 * 8 * 128], BF16, kind="ExternalInput")
    wk_re = nc.dram_tensor("wk_re", [128, 8 * 8 * 128], BF16, kind="ExternalInput")
    wv_raw = nc.dram_tensor("wv_raw", [D, D], BF16, kind="ExternalInput")
    wo_re = nc.dram_tensor("wo_re", [128, 8 * D], BF16, kind="ExternalInput")
    w1_re = nc.dram_tensor("w1_re", [128, 32 * 8 * 128], BF16, kind="ExternalInput")
    w2 = nc.dram_tensor("w2", [DFF, D], BF16, kind="ExternalInput")
    masksc = nc.dram_tensor("masksc", [Q, Q], F32, kind="ExternalInput")
    bqkvcol = nc.dram_tensor("bqkvcol", [128, 16], F32, kind="ExternalInput")
    bvc = nc.dram_tensor("bvc", [64, H], F32, kind="ExternalInput")
    b1col = nc.dram_tensor("b1col", [128, 32], F32, kind="ExternalInput")
    borow = nc.dram_tensor("borow", [128, D], F32, kind="ExternalInput")
    b2row = nc.dram_tensor("b2row", [128, D], F32, kind="ExternalInput")
    l1a = nc.dram_tensor("l1a", [128, D], F32, kind="ExternalInput")
    l1b = nc.dram_tensor("l1b", [128, D], F32, kind="ExternalInput")
    l2a = nc.dram_tensor("l2a", [128, D], F32, kind="ExternalInput")
    l2b = nc.dram_tensor("l2b", [128, D], F32, kind="ExternalInput")
    onesc_d = nc.dram_tensor("onesc_d", [65, 64], F32R, kind="ExternalInput")
    sel_d = nc.dram_tensor("sel_d", [65, 16 * 16], F32R, kind="ExternalInput")
    e16_d = nc.dram_tensor("e16_d", [16, 16 * 64], F32R, kind="ExternalInput")
    out_d = nc.dram_tensor("out", [Q, D], F32, kind="ExternalOutput")

    with tile.TileContext(nc) as tc, nc.allow_low_precision("bf16/fp32r pipeline"):
        with tc.tile_pool(name="consts", bufs=1) as consts, \
             tc.tile_pool(name="attp", bufs=1) as attp:
            mask_sb = consts.tile([128, 4, Q], F32, tag="mask")
            for t in range(4):
                nc.sync.dma_start(out=mask_sb[:, t, :],
                                  in_=masksc[t * 128:(t + 1) * 128, :])
            bqkv_sb = consts.tile([128, 16], F32, tag="bqkv")
            nc.sync.dma_start(out=bqkv_sb[:], in_=bqkvcol[:])
            bvc_sb = consts.tile([64, H], F32, tag="bvc")
            nc.sync.dma_start(out=bvc_sb[:], in_=bvc[:])
            b1c_sb = consts.tile([128, 32], F32, tag="b1c")
            nc.sync.dma_start(out=b1c_sb[:], in_=b1col[:])
            ver = consts.tile([1, 8], F32, tag="ver")
            nc.vector.memset(ver[:], 10.0)  # build version marker (cache key)
            sel_sb = consts.tile([65, 16 * 16], F32R, tag="sel")
            nc.sync.dma_start(out=sel_sb[:], in_=sel_d[:])
            e16_sb = consts.tile([16, 16 * 64], F32R, tag="e16")
            nc.sync.dma_start(out=e16_sb[:], in_=e16_d[:])

            attA = attp.tile([65, 8, Q], F32R, tag="attA")
            attB = attp.tile([65, 8, Q], F32R, tag="attB")
            att2AB = attp.tile([128, 8, Q], BF16, tag="att2AB")
            att2Bt = attp.tile([64, 8, Q], BF16, tag="att2Bt")

            # ---------------- Phase 1: QKV + attention ----------------
            with tc.tile_pool(name="work", bufs=2) as work, \
                 tc.tile_pool(name="qtp", bufs=1) as qtp, \
                 tc.tile_pool(name="ptp", bufs=4) as ptp, \
                 tc.tile_pool(name="ps_av", bufs=1, space="PSUM") as ps_av:

                def load_xt(kc, nm):
                    t = work.tile([128, 8, 512], BF16, tag="xt", name=f"xt{nm}")
                    for d in range(8):
                        nc.sync.dma_start(
                            out=t[:, d, :],
                            in_=xt[d * 128:(d + 1) * 128, kc * 512:(kc + 1) * 512])
                    return t

                def load_ones(kc, v_t):
                    for tt in range(4):
                        nc.sync.dma_start(
                            out=v_t[:, tt, :].rearrange(
                                "p (h c) -> p h c", c=65)[:, :, 64:65],
                            in_=onesg[kc * 512 + tt * 128: kc * 512 + (tt + 1) * 128, :])

                def attn_hp_ops(kc, hp, st_pool, cur_kt, cur_v):
                    avA = ps_av.tile([65, Q], F32, tag="avA", name=f"avA{kc}_{hp}")
                    avB = ps_av.tile([65, Q], F32, tag="avB", name=f"avB{kc}_{hp}")
                    pts = {}

                    def issue_st(t):
                        st = st_pool.tile([128, 2 * Q], F32, tag="st",
                                          name=f"st{kc}_{hp}_{t}")
                        nc.tensor.matmul(
                            st[:, 0:Q], cur_kt[0:64, hp, t * 128:(t + 1) * 128],
                            qt_sb[0:64, hp, :], start=True, stop=True)
                        nc.tensor.matmul(
                            st[:, Q:2 * Q], cur_kt[64:128, hp, t * 128:(t + 1) * 128],
                            qt_sb[64:128, hp, :], start=True, stop=True)
                        if kc == 3:
                            import dataclasses as _dc
                            m = mask_sb[:, t, :]
                            mb = _dc.replace(m, ap=[m.ap[0], [0, 2]] + m.ap[1:])
                            nc.vector.tensor_add(
                                st[:].rearrange("p (g q) -> p g q", g=2),
                                st[:].rearrange("p (g q) -> p g q", g=2),
                                mb)
                        pt = ptp.tile([128, 2 * Q], BF16, tag="pt",
                                      name=f"pt{kc}_{hp}_{t}")
                        nc.scalar.activation(pt[:], st[:], AF.Exp, scale=0.125)
                        pts[t] = pt

                    def issue_av(t):
                        pt = pts.pop(t)
                        nc.tensor.matmul(
                            avA[:], cur_v[:, t, (2 * hp) * 65:(2 * hp) * 65 + 65],
                            pt[:, 0:Q], start=(t == 0), stop=(t == 3),
                            skip_group_check=True)
                        nc.tensor.matmul(
                            avB[:],
                            cur_v[:, t, (2 * hp + 1) * 65:(2 * hp + 1) * 65 + 65],
                            pt[:, Q:2 * Q], start=(t == 0), stop=(t == 3),
                            skip_group_check=True)

                    def finish():
                        if kc == 0:
                            nc.scalar.activation(attA[:, hp, :], avA[:], AF.Copy)
                            nc.scalar.activation(attB[:, hp, :], avB[:], AF.Copy)
                        else:
                            nc.vector.tensor_add(attA[:, hp, :], attA[:, hp, :], avA[:])
                            nc.vector.tensor_add(attB[:, hp, :], attB[:, hp, :], avB[:])

                    return issue_st, issue_av, finish

                def attn_kc(kc, st_pool, cur_kt, cur_v, after_hp=None):
                    # Cross-head-pair pipeline: each hp's av2/av3 are delayed
                    # until after the next hp's st0/st1 so exp latency hides.
                    prev = None
                    for hp in range(8):
                        ist, iav, fin = attn_hp_ops(kc, hp, st_pool, cur_kt, cur_v)
                        ist(0)
                        ist(1)
                        if prev is not None:
                            prev[1](2)
                            prev[1](3)
                            prev[2]()
                        iav(0)
                        ist(2)
                        iav(1)
                        ist(3)
                        prev = (ist, iav, fin)
                        if after_hp is not None:
                            after_hp(hp)
                    prev[1](2)
                    prev[1](3)
                    prev[2]()

                qt_sb = qtp.tile([128, 8, Q], BF16, tag="qt")

                with tc.tile_pool(name="ps_qkv", bufs=2, space="PSUM") as ps_qkv, \
                     tc.tile_pool(name="ps_st", bufs=2, space="PSUM") as ps_st, \
                     tc.tile_pool(name="wres", bufs=1) as wres:

                    # resident qkv weights (bf16), phase-1 lifetime only
                    wk_sb = wres.tile([128, 8, 8, 128], BF16, tag="wk")
                    wq_sb = wres.tile([128, 8, 8, 128], BF16, tag="wq")
                    for f in range(8):
                        nc.sync.dma_start(out=wk_sb[:, f],
                                          in_=wk_re[:, f * 1024:(f + 1) * 1024])
                        nc.sync.dma_start(out=wq_sb[:, f],
                                          in_=wq_re[:, f * 1024:(f + 1) * 1024])
                    wv_sb = wres.tile([128, 8, 2, 512], BF16, tag="wv")
                    for d in range(8):
                        nc.sync.dma_start(out=wv_sb[:, d],
                                          in_=wv_raw[d * 128:(d + 1) * 128, :])

                    def produce_k_group(kc, xt_t, kt_t, kf):
                        ps = ps_qkv.tile([128, 512], F32, tag="qkv",
                                         name=f"psk{kc}_{kf}")
                        for d in range(8):
                            nc.tensor.matmul(ps[:], wk_sb[:, kf, d], xt_t[:, d, :],
                                             start=(d == 0), stop=(d == 7))
                        nc.vector.tensor_scalar_add(kt_t[:, kf, :], ps[:],
                                                    bqkv_sb[:, 8 + kf:9 + kf])

                    def produce_v_group(kc, xt_t, v_t, vc, tt):
                        ps = ps_qkv.tile([128, 512], F32, tag="qkv",
                                         name=f"psv{kc}_{vc}_{tt}")
                        for d in range(8):
                            nc.tensor.matmul(
                                ps[:], xt_t[:, d, tt * 128:(tt + 1) * 128],
                                wv_sb[:, d, vc], start=(d == 0), stop=(d == 7))
                        nc.scalar.activation(
                            v_t[:, tt, :].rearrange(
                                "p (h c) -> p h c", c=65)[:, vc * 8:(vc + 1) * 8, 0:64],
                            ps[:].rearrange("p (h c) -> p h c", c=64),
                            AF.Copy)

                    # Q projection from the own-chunk slots (1536:2048)
                    xt_q = load_xt(3, "q")
                    for qf in range(8):
                        ps = ps_qkv.tile([128, Q], F32, tag="qkv", name=f"psq{qf}")
                        for d in range(8):
                            nc.tensor.matmul(ps[:], wq_sb[:, qf, d], xt_q[:, d, :],
                                             start=(d == 0), stop=(d == 7))
                        nc.vector.tensor_scalar_add(qt_sb[:, qf, :], ps[:],
                                                    bqkv_sb[:, qf:qf + 1])

                    # K/V for kc=0 upfront
                    cur_xt = load_xt(0, "0")
                    cur_kt = work.tile([128, 8, 512], BF16, tag="kt", name="kt0")
                    cur_v = work.tile([128, 4, H * 65], BF16, tag="v", name="v0")
                    load_ones(0, cur_v)
                    for kf in range(8):
                        produce_k_group(0, cur_xt, cur_kt, kf)
                    for vc in range(2):
                        for tt in range(4):
                            produce_v_group(0, cur_xt, cur_v, vc, tt)

                    for kc in range(3):
                        nxt_xt = load_xt(kc + 1, str(kc + 1))
                        nxt_kt = work.tile([128, 8, 512], BF16, tag="kt",
                                           name=f"kt{kc + 1}")
                        nxt_v = work.tile([128, 4, H * 65], BF16, tag="v",
                                          name=f"v{kc + 1}")
                        load_ones(kc + 1, nxt_v)

                        def interleave(hp, kc=kc, nxt_xt=nxt_xt, nxt_kt=nxt_kt,
                                       nxt_v=nxt_v):
                            for g in (2 * hp, 2 * hp + 1):
                                if g < 8:
                                    produce_k_group(kc + 1, nxt_xt, nxt_kt, g)
                                else:
                                    vc, tt = divmod(g - 8, 4)
                                    produce_v_group(kc + 1, nxt_xt, nxt_v, vc, tt)

                        attn_kc(kc, ps_st, cur_kt, cur_v, after_hp=interleave)
                        cur_xt, cur_kt, cur_v = nxt_xt, nxt_kt, nxt_v

                # diagonal chunk: deeper pipeline, no production interleave
                with tc.tile_pool(name="ps_st3", bufs=3, space="PSUM") as ps_st3:
                    attn_kc(3, ps_st3, cur_kt, cur_v)

            # normalize: att2[f, q] = att[f, q] / r[q] + bv
            # extract all 16 r rows -> [16, 512], one wide reciprocal, broadcast
            with tc.tile_pool(name="ps_bc", bufs=3, space="PSUM") as ps_bc, \
                 tc.tile_pool(name="normp", bufs=1) as normp:
                r_all = ps_bc.tile([16, Q], F32, tag="rall", name="r_all")
                k = 0
                for hp in range(8):
                    for att, h in ((attA, 2 * hp), (attB, 2 * hp + 1)):
                        nc.tensor.matmul(r_all[:], sel_sb[:, h * 16:(h + 1) * 16],
                                         att[0:65, hp, :],
                                         start=(k == 0), stop=(k == 15))
                        k += 1
                rinv = normp.tile([16, Q], F32R, tag="rinv")
                nc.vector.reciprocal(rinv[:], r_all[:])
                for hp in range(8):
                    for att, att2, h in ((attA, att2AB[0:64], 2 * hp),
                                         (attB, att2Bt, 2 * hp + 1)):
                        bc = ps_bc.tile([64, Q], F32, tag="bc", name=f"bc{hp}_{h % 2}")
                        nc.tensor.matmul(bc[:], e16_sb[:, h * 64:(h + 1) * 64],
                                         rinv[:], start=True, stop=True)
                        nc.vector.tensor_mul(att2[:, hp, :], att[0:64, hp, :], bc[:])
                        nc.vector.tensor_scalar_add(att2[:, hp, :], att2[:, hp, :],
                                                    bvc_sb[:, h:h + 1])
                    # move B head to partitions 64..127 (only DMA crosses partitions)
                    nc.sync.dma_start(out=att2AB[64:128, hp, :], in_=att2Bt[:, hp, :])


            # ---------------- Phase 2: O-proj + residual + LN1 ----------------
            mid = tc.alloc_tile_pool(name="mid", bufs=1)
            xln1 = mid.tile([128, 4, D], F32, tag="xln1")
            xln1t = mid.tile([128, 8, Q], BF16, tag="xln1t")
            with tc.tile_pool(name="op2", bufs=2) as op2, \
                 tc.tile_pool(name="wop", bufs=1) as wop, \
                 tc.tile_pool(name="bc2", bufs=4) as bc2, \
                 tc.tile_pool(name="ident", bufs=1) as identp, \
                 tc.tile_pool(name="ps_o", bufs=4, space="PSUM") as ps_o, \
                 tc.tile_pool(name="ps_t", bufs=2, space="PSUM") as ps_t:
                from concourse.masks import make_identity
                ident = identp.tile([128, 128], F32, tag="ident")
                make_identity(nc, ident[:])
                bo_sb = bc2.tile([128, D], F32, tag="row")
                nc.sync.dma_start(out=bo_sb[:], in_=borow[:])
                l1a_sb = bc2.tile([128, D], F32, tag="row")
                nc.sync.dma_start(out=l1a_sb[:], in_=l1a[:])
                l1b_sb = bc2.tile([128, D], F32, tag="row")
                nc.sync.dma_start(out=l1b_sb[:], in_=l1b[:])
                b2f_sb = bc2.tile([128, D], F32, tag="row")
                nc.sync.dma_start(out=b2f_sb[:], in_=b2row[:])
                wo_t = {}
                for f in range(8):
                    for n in range(2):
                        w = wop.tile([128, 512], BF16, tag=f"wo{f}_{n}", name=f"wo{f}_{n}")
                        nc.sync.dma_start(
                            out=w[:], in_=wo_re[:, f * D + n * 512: f * D + (n + 1) * 512])
                        wo_t[(f, n)] = w
                def issue_transposes(qt):
                    for d in range(8):
                        pst = ps_t.tile([128, 128], F32, tag="pst",
                                        name=f"pst{qt}_{d}")
                        nc.tensor.transpose(pst[:], xln1[:, qt, d * 128:(d + 1) * 128],
                                            ident[:])
                        nc.scalar.activation(
                            xln1t[:, d, qt * 128:(qt + 1) * 128], pst[:], AF.Copy)

                prev_qt = None
                for qt in range(4):
                    pso = [ps_o.tile([128, 512], F32, tag="pso", name=f"pso{qt}_{i}")
                           for i in range(2)]
                    for f in range(8):
                        for n in range(2):
                            nc.tensor.matmul(
                                pso[n][:],
                                att2AB[:, f, qt * 128:(qt + 1) * 128],
                                wo_t[(f, n)][:],
                                start=(f == 0), stop=(f == 7),
                                skip_group_check=True)
                    xc_t = op2.tile([128, D], F32, tag="xct")
                    nc.sync.dma_start(out=xc_t[:], in_=xc[qt * 128:(qt + 1) * 128, :])
                    y = op2.tile([128, D], F32, tag="y")
                    for n in range(2):
                        nc.vector.tensor_add(y[:, n * 512:(n + 1) * 512],
                                             pso[n][:], xc_t[:, n * 512:(n + 1) * 512])
                    nc.vector.tensor_add(y[:], y[:], bo_sb[:])
                    _layernorm(nc, op2, y, xln1[:, qt, :], l1a_sb, l1b_sb)
                    nc.vector.tensor_add(xln1[:, qt, :], xln1[:, qt, :], b2f_sb[:])
                    if prev_qt is not None:
                        issue_transposes(prev_qt)
                    prev_qt = qt
                issue_transposes(3)

            # ---------------- Phase 3: MLP + LN2 + out ----------------
            with tc.tile_pool(name="ht", bufs=1) as htp, \
                 tc.tile_pool(name="mw", bufs=3) as mw, \
                 tc.tile_pool(name="bc3", bufs=4) as bc3, \
                 tc.tile_pool(name="zt", bufs=1) as ztp, \
                 tc.tile_pool(name="op3", bufs=3) as op3, \
                 tc.tile_pool(name="ps_h", bufs=2, space="PSUM") as ps_h, \
                 tc.tile_pool(name="ps_m", bufs=1, space="PSUM") as ps_m:
                ht = htp.tile([128, 32, Q], BF16, tag="ht")
                for t in range(32):
                    wslab = mw.tile([128, 8, 128], BF16, tag="w1s", bufs=6)
                    nc.sync.dma_start(out=wslab[:],
                                      in_=w1_re[:, t * 1024:(t + 1) * 1024])
                    ps = ps_h.tile([128, Q], F32, tag="psh")
                    for d in range(8):
                        nc.tensor.matmul(ps[:], wslab[:, d, :], xln1t[:, d, :],
                                         start=(d == 0), stop=(d == 7))
                    nc.vector.tensor_scalar(out=ht[:, t, :], in0=ps[:],
                                            scalar1=b1c_sb[:, t:t + 1], scalar2=0.0,
                                            op0=ALU.add, op1=ALU.max)
                l2a_sb = bc3.tile([128, D], F32, tag="row3")
                nc.sync.dma_start(out=l2a_sb[:], in_=l2a[:])
                l2b_sb = bc3.tile([128, D], F32, tag="row3")
                nc.sync.dma_start(out=l2b_sb[:], in_=l2b[:])
                z = [ztp.tile([128, D], F32, tag=f"z{qt}", name=f"z{qt}")
                     for qt in range(4)]
                for n in range(2):
                    psm = [ps_m.tile([128, 512], F32, tag=f"psm{qt}",
                                     name=f"psm{n}_{qt}") for qt in range(4)]
                    for t in range(32):
                        w2t = mw.tile([128, 512], BF16, tag="w2t", bufs=8,
                                      name=f"w2t{n}_{t}")
                        nc.sync.dma_start(
                            out=w2t[:],
                            in_=w2[t * 128:(t + 1) * 128, n * 512:(n + 1) * 512])
                        for qt in range(4):
                            nc.tensor.matmul(
                                psm[qt][:],
                                ht[:, t, qt * 128:(qt + 1) * 128],
                                w2t[:], start=(t == 0), stop=(t == 31))
                    for qt in range(4):
                        nc.vector.tensor_add(z[qt][:, n * 512:(n + 1) * 512],
                                             psm[qt][:],
                                             xln1[:, qt, n * 512:(n + 1) * 512])
                        if n == 1:
                            o_sb = op3.tile([128, D], F32, tag="osb",
                                            name=f"osb{qt}")
                            _layernorm(nc, op3, z[qt], o_sb[:], l2a_sb, l2b_sb)
                            nc.sync.dma_start(out=out_d[qt * 128:(qt + 1) * 128, :],
                                              in_=o_sb[:])
            mid.release()

    nc.finalize()
    return nc


def _layernorm(nc, pool, y, out_ap, a_sb, b_sb):
    """out = (y - mean) / (sqrt(unbiased var) + EPS) * a + b, per row of 1024."""
    stats = pool.tile([128, 2, 6], F32, tag="lnstats")
    for g in range(2):
        nc.vector.bn_stats(out=stats[:, g, :], in_=y[:, g * 512:(g + 1) * 512])
    mv = pool.tile([128, 2], F32, tag="lnmv")
    nc.vector.bn_aggr(out=mv[:], in_=stats[:])
    sd = pool.tile([128, 1], F32, tag="lnsd")
    nc.scalar.activation(sd[:], mv[:, 1:2], AF.Sqrt, scale=float(D) / (D - 1))
    nc.vector.tensor_scalar_add(sd[:], sd[:], EPS)
    nc.vector.reciprocal(sd[:], sd[:])
    nc.vector.tensor_scalar(out=out_ap, in0=y[:], scalar1=mv[:, 0:1], scalar2=sd[:],
                            op0=ALU.subtract, op1=ALU.mult)
    nc.vector.tensor_mul(out_ap, out_ap, a_sb[:])
    nc.vector.tensor_add(out_ap, out_ap, b_sb[:])


_SEL = np.zeros((65, 16 * 16), np.float32)
for _h in range(16):
    _SEL[64, _h * 16 + _h] = 1.0
_E16 = np.zeros((16, 16 * 64), np.float32)
for _h in range(16):
    _E16[_h, _h * 64:(_h + 1) * 64] = 1.0

_NC = None


def _get_nc():
    global _NC
    if _NC is None:
        _NC = _build_nc()
    return _NC


def kernel(x, mask, Wqkv, bqkv, Wo, bo, W1, b1, W2, b2, ln1_a, ln1_b, ln2_a, ln2_b):
    x = np.asarray(x, np.float32)
    Wqkv = np.asarray(Wqkv, np.float32)
    bqkv = np.asarray(bqkv, np.float32)
    Wo = np.asarray(Wo, np.float32)
    bo = np.asarray(bo, np.float32)
    W1 = np.asarray(W1, np.float32)
    b1 = np.asarray(b1, np.float32)
    W2 = np.asarray(W2, np.float32)
    b2 = np.asarray(b2, np.float32)
    ln1_a = np.asarray(ln1_a, np.float32)
    ln1_b = np.asarray(ln1_b, np.float32)
    ln2_a = np.asarray(ln2_a, np.float32)
    ln2_b = np.asarray(ln2_b, np.float32)
    bf = ml_dtypes.bfloat16

    wq_re = np.ascontiguousarray(
        Wqkv[:, :D].reshape(8, 128, 8, 128).transpose(1, 2, 0, 3).reshape(128, -1)
    ).astype(bf)
    wk_re = np.ascontiguousarray(
        Wqkv[:, D:2 * D].reshape(8, 128, 8, 128).transpose(1, 2, 0, 3).reshape(128, -1)
    ).astype(bf)
    wv_raw = np.ascontiguousarray(Wqkv[:, 2 * D:]).astype(bf)
    wo_re = np.ascontiguousarray(
        Wo.reshape(8, 2, 64, D).transpose(1, 2, 0, 3).reshape(128, -1)).astype(bf)
    w1_re = np.ascontiguousarray(
        W1.reshape(8, 128, 32, 128).transpose(1, 2, 0, 3).reshape(128, -1)).astype(bf)
    bqkvcol = np.ascontiguousarray(bqkv[:2 * D].reshape(16, 128).T)
    bvc = np.ascontiguousarray(bqkv[2 * D:].reshape(H, 64).T)
    b1col = np.ascontiguousarray(b1.reshape(32, 128).T)
    row = lambda v: np.ascontiguousarray(np.broadcast_to(v, (128, D)))
    s = np.arange(Q)
    masksc_np = np.where(s[:, None] <= s[None, :], 0.0, NEG).astype(np.float32)

    shared = dict(
        wq_re=wq_re, wk_re=wk_re, wv_raw=wv_raw, wo_re=wo_re, w1_re=w1_re,
        w2=np.ascontiguousarray(W2).astype(bf), masksc=masksc_np, bqkvcol=bqkvcol,
        bvc=bvc, b1col=b1col, borow=row(bo), b2row=row(b2),
        l1a=row(ln1_a), l1b=row(ln1_b), l2a=row(ln2_a), l2b=row(ln2_b),
        onesc_d=np.ones((65, 64), np.float32),
        sel_d=_SEL, e16_d=_E16,
    )

    in_maps = []
    for c in range(NCORES):
        b, j = divmod(c, 4)
        pref = 512 * j
        xtc = np.zeros((D, P), np.float32)
        if pref:
            xtc[:, :pref] = x[b, :pref].T
        xtc[:, 1536:] = x[b, pref:pref + Q].T
        onesg = np.zeros((P, H), np.float32)
        onesg[:pref] = 1.0
        onesg[1536:] = 1.0
        in_maps.append(dict(shared, xt=xtc.astype(bf),
                            xc=np.ascontiguousarray(x[b, pref:pref + Q]),
                            onesg=onesg.astype(bf)))

    res = run_bass_kernel_spmd(_get_nc(), in_maps, list(range(NCORES)))
    kernel.LAST_RESULT = res
    out = np.empty((B, L, D), np.float32)
    for c in range(NCORES):
        b, j = divmod(c, 4)
        out[b, 512 * j:512 * (j + 1)] = res.results[c]["out"]
    return out
